# revision 26
# baseline (speedup 1.0000x reference)
"""Trainium2 Bass kernel v10 for nn_AaD_MAPU (retrieval kNN shortlist).

Drain-bound design. The PE computes the full fp8 distance matrix
(512 q x 12544 cols per core) at full clock (~21us); the binding
constraint is PSUM-exit bandwidth: every dot value must leave PSUM
through Act (0.83 ns/v) or DVE (1.04 ns/v); GPSIMD has no PSUM port,
DMA cannot read PSUM. Two paths per half-tile [128, 2, 512]:

  B  : DVE tensor_reduce from PSUM -> per-128-col bucket maxes (fp32)
  A2 : Act exp(psum/16) -> fp8 tile, DMA ships it to the host.
       exp-companding keeps ~bf16-class ranking precision at the top
       of the dot range in 1 byte, halving ship bandwidth.

Input stream: whole fbt (fnt + bank shard, fp8) is SBUF-resident; 13
chunked DMAs (2 tiles each) issued upfront on SP so the DMA engine
streams back-to-back. bred ships in two pieces (bulk mid-kernel, tiny tail).

Host: bucket shortlist (128-col bucket maxes from both paths), exact
fp32 re-rank of top bucket members, fp64 loss.
"""

from contextlib import ExitStack

import numpy as np

import concourse.bass as bass
import concourse.tile as tile
from concourse import bacc, mybir
from concourse.bass_utils import run_bass_kernel_spmd

B, D, N, C, K = 512, 512, 100000, 64, 5
EPS = 1e-12
NCORES = 8
NSHARD = 12544
NPAD = NSHARD * NCORES
FSCALE = 16.0

NT = 25                      # tiles: 24 x 512 + 1 x 256
TILE_W = [512] * 24 + [256]
N_WARMUP = 7
N_A2 = 26                    # halves shipped via Act exp->fp8; rest DVE
BRED_SPLIT = 16              # bred tiles < 16 ship mid-kernel
SLABS = [3, 3, 3, 3, 3, 3, 3, 3, 2]   # A2 halves per ship DMA
CHUNK_TILES = 2              # input tiles per DMA chunk
N_ACT_TAIL = 0               # how many trailing slabs issue from the Act queue
BREDB_ON_ACT = False         # final bred piece issues from the Act queue
BRED3 = False                # third bred piece: tile 24 alone ships last (tiny)

_F32 = mybir.dt.float32
_FP8 = mybir.dt.float8e4

_cache = {}


def _assign_paths():
    """50 halves -> 'B' | '2'. Tile 24 halves (small) and the final full
    half forced B; Bresenham-spread otherwise."""
    n = 2 * NT
    forced_b = {2 * 24, 2 * 24 + 1, 2 * 23 + 1}
    quota = {"B": n - N_A2 - len(forced_b), "2": N_A2}
    issued = {"B": 0, "2": 0}
    out = []
    for i in range(n):
        if i in forced_b:
            out.append("B")
            continue
        cand = [p for p in ("B", "2") if issued[p] < quota[p]]
        p = min(cand, key=lambda q: (issued[q] + 0.5) / quota[q])
        issued[p] += 1
        out.append(p)
    return out


def _build_module():
    nc = bacc.Bacc("TRN2", target_bir_lowering=False, debug=False,
                   num_devices=NCORES)
    # columns 0:512 = f_norm.T * FSCALE, columns 512: = bank shard.T
    fbt_d = nc.dram_tensor("fbt", [D, B + NSHARD], _FP8,
                           kind="ExternalInput").ap()
    raw_out = nc.dram_tensor("raw_out", [128, max(N_A2, 1), 2, 512], _FP8,
                             kind="ExternalOutput").ap()
    # bred[p, t, h, bi, g]: B-path bucket maxes (fp32, units of 16*d)
    bred_out = nc.dram_tensor("bred_out", [128, NT, 2, 2, 4], _F32,
                              kind="ExternalOutput").ap()

    paths = _assign_paths()

    with tile.TileContext(nc) as tc, ExitStack() as ctx:
        const = ctx.enter_context(tc.tile_pool(name="const", bufs=1))
        dp_pool = ctx.enter_context(tc.tile_pool(name="dp", bufs=4, space="PSUM"))

        # PE warm-up + Exp act-table preload during the initial DMA wait.
        wu_sb = const.tile([128, 512], _F32)
        nc.gpsimd.memset(wu_sb[:], 0.0)
        wu_act = const.tile([128, 1], _F32, name="wu_act")
        nc.scalar.activation(out=wu_act[:], in_=wu_sb[:, 0:1],
                             func=mybir.ActivationFunctionType.Exp)
        wu_ps = dp_pool.tile([128, 2, 512], _F32, tag="dp")
        wu_r = wu_sb[:].bitcast(_FP8).rearrange("p (c j) -> p c j", c=4)
        for _ in range(N_WARMUP):
            nc.tensor.matmul(wu_ps[:, 0], lhsT=wu_r[:, 0:2, :128], rhs=wu_r[:, 0:2],
                             start=True, stop=True,
                             perf_mode=mybir.MatmulPerfMode.DoubleRow)
        wu_reuse = [wu_ps]

        # SBUF-resident fbt: ch0 = fnt + leading tiles, then tile groups.
        # All input DMAs issued upfront on SP.
        if CHUNK_TILES == 2:
            chw = [1024] * 12 + [768]
        else:
            chw = [2560] + [2048] * 5 + [256]
        chunks = []
        j0 = 0
        for k, w in enumerate(chw):
            ch = const.tile([128, 4, w], _FP8, name=f"ch{k}")
            nc.sync.dma_start(
                ch[:], fbt_d[:, j0:j0 + w].rearrange("(c p) j -> p c j", p=128))
            chunks.append(ch)
            j0 += w

        fnt_sb = chunks[0][:, :, 0:512]

        csum = np.cumsum([0] + chw).tolist()

        def tile_rhs(t):
            # tile t = bank cols [512t, 512t+512) = fbt cols 512+512t ..
            j = 512 + 512 * t
            k = next(i for i in range(len(chw))
                     if csum[i] <= j and j + TILE_W[t] <= csum[i + 1])
            return chunks[k][:, :, j - csum[k]:j - csum[k] + TILE_W[t]]

        bredA = const.tile([128, BRED_SPLIT, 2, 2, 4], _F32, name="bredA")
        nb = (NT - 1 if BRED3 else NT) - BRED_SPLIT
        bredB = const.tile([128, nb, 2, 2, 4], _F32, name="bredB")
        bredC = (const.tile([128, 1, 2, 2, 4], _F32, name="bredC")
                 if BRED3 else None)
        if SLABS is None:
            tmps = [const.tile([128, 2, 512], _FP8, name=f"tmp{i}")
                    for i in range(N_A2)]
            slab_edges = []
        else:
            assert sum(SLABS) == N_A2
            tmps = [const.tile([128, w, 2, 512], _FP8, name=f"tmp{i}")
                    for i, w in enumerate(SLABS)]
            slab_edges = list(np.cumsum(SLABS))

        ti = 0   # raw slot index
        si = 0   # slab index
        s0 = 0   # current slab start
        bred_shipped = False
        for t in range(NT):
            W = TILE_W[t]
            fbt = tile_rhs(t)
            for h in range(2):
                p = paths[2 * t + h]
                if wu_reuse:
                    dp = wu_reuse.pop()
                else:
                    dp = dp_pool.tile([128, 2, 512], _F32, tag="dp")
                for bi in range(2):
                    bc = 2 * h + bi
                    for dc in range(2):
                        nc.tensor.matmul(
                            dp[:, bi, :W],
                            lhsT=fnt_sb[:, 2 * dc:2 * dc + 2,
                                        bc * 128:(bc + 1) * 128],
                            rhs=fbt[:, 2 * dc:2 * dc + 2, :W],
                            start=(dc == 0), stop=(dc == 1),
                            perf_mode=mybir.MatmulPerfMode.DoubleRow,
                        )
                if p == "B":
                    g = W // 128
                    if t < BRED_SPLIT:
                        bslot = bredA[:, t, h]
                    elif BRED3 and t == NT - 1:
                        bslot = bredC[:, 0, h]
                    else:
                        bslot = bredB[:, t - BRED_SPLIT, h]
                    nc.vector.tensor_reduce(
                        out=bslot[:, :, :g],
                        in_=dp[:, :, :W].rearrange("p a (g c) -> p a g c", c=128),
                        axis=mybir.AxisListType.X, op=mybir.AluOpType.max)
                else:
                    if SLABS is None:
                        tm = tmps[ti][:, :, :W]
                    else:
                        tm = tmps[si][:, ti - s0, :, :W]
                    nc.scalar.activation(out=tm, in_=dp[:, :, :W],
                                         func=mybir.ActivationFunctionType.Exp,
                                         scale=1.0 / FSCALE)
                    if SLABS is None:
                        nc.sync.dma_start(raw_out[:, ti, :, :W], tm)
                        ti += 1
                    else:
                        ti += 1
                        if ti in slab_edges:
                            late = si >= len(SLABS) - N_ACT_TAIL
                            eng = nc.scalar if late else nc.sync
                            eng.dma_start(raw_out[:, s0:ti], tmps[si][:])
                            s0 = ti
                            si += 1
            if t == BRED_SPLIT + 1 and not bred_shipped:
                nc.sync.dma_start(bred_out[:, :BRED_SPLIT], bredA[:])
                bred_shipped = True
            if BRED3 and t == NT - 2:
                # tiles 16..23 complete; ship before the final slab/tile
                nc.sync.dma_start(bred_out[:, BRED_SPLIT:NT - 1], bredB[:])

        if BRED3:
            nc.sync.dma_start(bred_out[:, NT - 1:], bredC[:])
        else:
            eng = nc.scalar if BREDB_ON_ACT else nc.sync
            eng.dma_start(bred_out[:, BRED_SPLIT:], bredB[:])

    nc.compile()
    return nc


def _get_module():
    if "nc" not in _cache:
        _cache["nc"] = _build_module()
    return _cache["nc"]


def _host_tables():
    if "tables" in _cache:
        return _cache["tables"]
    paths = _assign_paths()
    fine = {0: [], 1: []}   # per h: list of (t, src, slot)
    ti = 0
    for t in range(NT):
        for h in range(2):
            if paths[2 * t + h] == "B":
                fine[h].append((t, "B", 0))
            else:
                fine[h].append((t, "2", ti))
                ti += 1
    _cache["tables"] = (paths, fine)
    return _cache["tables"]


def kernel(features, predictions, fea_bank, score_bank, trg_idx):
    features = np.asarray(features, dtype=np.float32)
    predictions = np.asarray(predictions, dtype=np.float32)
    fea_bank = np.asarray(fea_bank, dtype=np.float32)
    score_bank = np.asarray(score_bank, dtype=np.float32)
    trg_idx = np.asarray(trg_idx, dtype=np.int32)

    sm = predictions - predictions.max(axis=1, keepdims=True)
    np.exp(sm, out=sm)
    sm /= sm.sum(axis=1, keepdims=True)
    nrm = np.maximum(np.sqrt((features * features).sum(axis=1, keepdims=True)),
                     EPS)
    f_norm = features / nrm

    fbp = np.zeros((NPAD, D), dtype=np.float32)
    fbp[:N] = fea_bank
    fbp[trg_idx] = f_norm
    sb = score_bank.copy()
    sb[trg_idx] = sm

    import ml_dtypes
    fp8 = ml_dtypes.float8_e4m3
    fnt_cols = (f_norm.T * FSCALE).astype(np.float32)

    nc = _get_module()
    in_maps = [
        {"fbt": np.ascontiguousarray(np.concatenate(
            [fnt_cols, fbp[c * NSHARD:(c + 1) * NSHARD].T],
            axis=1)).astype(fp8)}
        for c in range(NCORES)
    ]
    res = run_bass_kernel_spmd(nc, in_maps, core_ids=list(range(NCORES)))

    paths, fine = _host_tables()

    breds = [r["bred_out"] for r in res.results]                   # 16*d fp32
    raws = [r["raw_out"].astype(np.float32) for r in res.results]  # exp(d) fp8

    TOPF = 16   # buckets kept per query

    rows_h = [None, None]
    for h in range(2):
        fl = fine[h]
        nf = len(fl)
        fv = np.full((128, 2, NCORES, nf, 4), -np.inf, np.float32)
        with np.errstate(divide="ignore"):
            for c in range(NCORES):
                for fi, (t, src, slot) in enumerate(fl):
                    g = TILE_W[t] // 128
                    if src == "B":
                        fv[:, :, c, fi, :g] = breds[c][:, t, h, :, :g] / FSCALE
                    else:
                        v = raws[c][:, slot, :, :TILE_W[t]]
                        v = v.reshape(128, 2, g, 128).max(axis=3)
                        fv[:, :, c, fi, :g] = np.log(v)
        fbase = np.empty((NCORES, nf, 4), np.int64)
        for c in range(NCORES):
            for fi, (t, src, slot) in enumerate(fl):
                for g in range(4):
                    fbase[c, fi, g] = c * NSHARD + 512 * t + 128 * g
        fvf = fv.reshape(128, 2, NCORES * nf * 4)
        fbf = fbase.reshape(NCORES * nf * 4)
        selF = np.argpartition(-fvf, TOPF, axis=2)[:, :, :TOPF]
        rows_h[h] = (fbf[selF][..., None] + np.arange(128, dtype=np.int64)
                     ).reshape(128, 2, TOPF * 128)

    ncand = TOPF * 128
    rows_all = np.zeros((B, ncand), np.int64)
    for h in range(2):
        for bi in range(2):
            q0 = (2 * h + bi) * 128
            rows_all[q0:q0 + 128] = rows_h[h][:, bi]

    # ---- exact re-rank -----------------------------------------------------
    dots = np.empty((B, ncand), np.float32)
    CH = 64
    for q0 in range(0, B, CH):
        rr = rows_all[q0:q0 + CH]
        vec = fbp[rr.reshape(-1)].reshape(CH, ncand, D)
        dots[q0:q0 + CH] = np.einsum("qkd,qd->qk", vec,
                                     f_norm[q0:q0 + CH], optimize=True)
    dots = np.where(rows_all < N, dots, np.float32(-np.inf))

    # top-6 rows, ties by lower row id (match jax top_k); buckets are
    # disjoint so no dedupe needed.
    order = np.lexsort((rows_all, -dots), axis=1)[:, :K + 1]
    top_idx = np.take_along_axis(rows_all, order, axis=1)

    idx_near = top_idx[:, 1:K + 1]
    score_near = sb[idx_near].astype(np.float64)
    kl = score_near * (np.log(score_near) - sm[:, None, :].astype(np.float64))
    loss = kl.sum(axis=(1, 2)).mean()

    s64 = sm.astype(np.float64)
    neg_pred = (np.square(s64.sum(axis=0)).sum()
                - np.square(s64).sum()) / B

    return np.float32(loss + neg_pred)
